# revision 1
# baseline (speedup 1.0000x reference)
"""Trainium2 Bass kernel for nn_Cross_attention_2 (sparse_attention).

Math (B=1, C=32, D=36, H=W=48, P=9):
  xc = conv1x1(x, W_img, b_img)            # per-voxel channel mix
  v  = unfold(xc)                          # (C, L=1024, 81) non-overlapping 9x9 patches
  px = LeakyReLU(v @ (W2@W1)^T + bias)     # the two Linears collapse to A = W2@W1
  att[c] = px[c] @ py[c]^T / 81            # (C, 1024, 1024)

Sharding: channels C=32 split across 8 cores (4 each). Params replicated
(per-core slices precomputed on host). Each core reads full x, y.

Per-core device pipeline (fp32 data, fp32r PE mode; all matmul outputs at
PSUM partition base 0 — fp32r codegen requires it):
  conv:      3 accumulating zero-padded block-diag matmuls (K=128/128/32)
             -> xc_sb (37, 4, 2304): rows kd*4+o, row 36 = 1.0 (bias row)
  transform: unfold folded into strided rhs APs; 9 kw-accumulation passes per
             output tile; combined weight TM includes channel select + A + bias
  att:       pxT/pyT kept as (81, 1024); out tiles (128, 512) per matmul
"""

import sys

sys.path.insert(0, "/opt/trn_rl_repo")

import contextlib
import os

import numpy as np

import concourse.bass as bass  # noqa: F401
import concourse.tile as tile
from concourse import bacc, mybir
from concourse.bass_utils import run_bass_kernel_spmd

P = 9
P2 = 81
C = 32
D = 36
HWF = 2304
ND = 4  # pd blocks (D/9)
L = 1024
N_CORES = 8
CPC = 4  # channels per core

F32 = mybir.dt.float32
F32R = mybir.dt.float32r

_CACHE = {}
last_results = None  # BassKernelResults of the most recent run (for test.py)

_HW_CHUNKS = [(0, 512), (512, 512), (1024, 512), (1536, 512), (2048, 256)]
_KD_PASSES = [(0, 4), (4, 4), (8, 1)]  # (kd0, nkd) conv passes


def _build():
    if "nc" in _CACHE:
        return _CACHE["nc"]

    nc = bacc.Bacc("TRN2", target_bir_lowering=False, debug=False,
                   num_devices=N_CORES)
    x_d = nc.dram_tensor("x", (C, D, HWF), F32R, kind="ExternalInput").ap()
    y_d = nc.dram_tensor("y", (C, D, HWF), F32R, kind="ExternalInput").ap()
    # wblk: (128, 216) = conv lhsT for (t in 2) x (pass i in 3), 36 cols each
    wblk_d = nc.dram_tensor("wblk", (128, 216), F32R, kind="ExternalInput").ap()
    # tm: (37, 2*4*9*81) combined transform weights in SBUF layout
    tm_d = nc.dram_tensor("tm", (37, 2 * CPC * P * P2), F32R,
                          kind="ExternalInput").ap()
    ones_d = nc.dram_tensor("ones", (1, ND * HWF), F32R,
                            kind="ExternalInput").ap()
    att_d = nc.dram_tensor("att", (CPC, L, L), F32, kind="ExternalOutput").ap()

    with tile.TileContext(nc) as tc:
        with contextlib.ExitStack() as ctx:
            consts = ctx.enter_context(tc.tile_pool(name="consts", bufs=1))
            xbp = ctx.enter_context(tc.tile_pool(name="xb", bufs=3))
            xbp2 = ctx.enter_context(tc.tile_pool(name="xb2", bufs=1))
            tmpp = ctx.enter_context(tc.tile_pool(name="tmp", bufs=2))
            outp = ctx.enter_context(tc.tile_pool(name="outp", bufs=2))
            cps = ctx.enter_context(tc.tile_pool(name="cps", bufs=2, space="PSUM"))
            tps = ctx.enter_context(tc.tile_pool(name="tps", bufs=2, space="PSUM"))
            aps = ctx.enter_context(tc.tile_pool(name="aps", bufs=3, space="PSUM"))

            wb_sb = consts.tile([128, 216], F32R, tag="wb")
            nc.sync.dma_start(out=wb_sb[:, :], in_=wblk_d[:, :])
            tm_sb = consts.tile([37, 2 * CPC * P * P2], F32R, tag="tm")
            nc.sync.dma_start(out=tm_sb[:, :], in_=tm_d[:, :])
            tm_v = tm_sb.rearrange("p (t c kw j) -> p t c kw j", t=2, c=CPC, kw=P)

            xc_sb = []
            px_sb = []
            for t in range(2):
                xt = consts.tile([37, ND, HWF], F32R, tag=f"xc{t}")
                nc.sync.dma_start(
                    out=xt[36:37, :, :],
                    in_=ones_d.rearrange("p (d h) -> p d h", d=ND))
                xc_sb.append(xt)
                px_sb.append([consts.tile([P2, L], F32R, tag=f"px{t}{c}",
                                          name=f"px{t}{c}")
                              for c in range(CPC)])

            for t in range(2):
                src = x_d if t == 0 else y_d
                for pd in range(ND):
                    xbs = []
                    for i, (kd0, nkd) in enumerate(_KD_PASSES):
                        kp = 32 * nkd
                        pool = xbp if nkd == 4 else xbp2
                        xb = pool.tile([kp, HWF], F32R, tag=f"xb{min(i, 1)}",
                                       name=f"xb{min(i, 1)}")
                        rows = src[:, pd * P + kd0: pd * P + kd0 + nkd, :]
                        nc.sync.dma_start(out=xb[:, :],
                                          in_=rows.transpose([1, 0, 2]))
                        xbs.append(xb)
                    for h0, hn in _HW_CHUNKS:
                        ps = cps.tile([36, 512], F32, tag="cps")
                        for i, (kd0, nkd) in enumerate(_KD_PASSES):
                            kp = 32 * nkd
                            lhs = wb_sb[0:kp,
                                        (t * 3 + i) * 36: (t * 3 + i + 1) * 36]
                            nc.tensor.matmul(
                                ps[:, :hn], lhs, xbs[i][:, h0: h0 + hn],
                                start=(i == 0), stop=(i == 2))
                        dst = xc_sb[t][0:36, pd, h0: h0 + hn]
                        if (h0 // 512) % 2 == 0:
                            nc.vector.tensor_copy(out=dst, in_=ps[:, :hn])
                        else:
                            nc.scalar.copy(out=dst, in_=ps[:, :hn])

                # transform: z = sum_kw TM[t,c,kw].T @ xc[:, :, kw::9]
                for c in range(CPC):
                    for ch in range(2):  # l-chunks of 512 (pd pairs)
                        zp = tps.tile([P2, 512], F32, tag="tps")
                        for kw in range(P):
                            rhs = xc_sb[t][:, 2 * ch: 2 * ch + 2, kw:HWF:P]
                            nc.tensor.matmul(
                                zp[:, :], tm_v[:, t, c, kw, :], rhs,
                                start=(kw == 0), stop=(kw == P - 1))
                        # LeakyReLU(z) = max(0.2*z, z)
                        zm = tmpp.tile([P2, 512], F32, tag="zm")
                        nc.scalar.mul(zm[:, :], zp[:, :], 0.2)
                        nc.vector.tensor_tensor(
                            out=px_sb[t][c][:, ch * 512: ch * 512 + 512],
                            in0=zp[:, :], in1=zm[:, :],
                            op=mybir.AluOpType.max)

            # att[c] = pxT[c].T @ pyT[c]
            for c in range(CPC):
                for m in range(8):  # l1 chunks of 128
                    ob = outp.tile([128, L], F32, tag="ob")
                    for nch in range(2):  # l2 chunks of 512
                        ap_ = aps.tile([128, 512], F32, tag="aps")
                        nc.tensor.matmul(
                            ap_[:, :],
                            px_sb[0][c][:, m * 128: m * 128 + 128],
                            px_sb[1][c][:, nch * 512: nch * 512 + 512],
                            start=True, stop=True)
                        dst = ob[:, nch * 512: nch * 512 + 512]
                        if nch % 2 == 0:
                            nc.vector.tensor_copy(out=dst, in_=ap_[:, :])
                        else:
                            nc.scalar.copy(out=dst, in_=ap_[:, :])
                    nc.sync.dma_start(
                        out=att_d[c, m * 128: m * 128 + 128, :], in_=ob[:, :])

    nc.compile()
    _CACHE["nc"] = nc
    return nc


def _host_prep(x, y, W_img, b_img, W_fea, b_fea, W1, W2):
    """Build per-core wblk / tm arrays. Returns in_maps list."""
    x = np.ascontiguousarray(np.asarray(x, np.float32).reshape(C, D, HWF))
    y = np.ascontiguousarray(np.asarray(y, np.float32).reshape(C, D, HWF))
    W_img = np.asarray(W_img, np.float32)
    b_img = np.asarray(b_img, np.float32)
    W_fea = np.asarray(W_fea, np.float32)
    b_fea = np.asarray(b_fea, np.float32)
    A = np.asarray(W2, np.float32) @ np.asarray(W1, np.float32)  # (81, 81)
    rowsum = A.sum(axis=1)  # (81,)
    ones = np.ones((1, ND * HWF), np.float32)

    in_maps = []
    for r in range(N_CORES):
        Wl = [W_img[r * CPC:(r + 1) * CPC, :], W_fea[r * CPC:(r + 1) * CPC, :]]
        bl = [b_img[r * CPC:(r + 1) * CPC], b_fea[r * CPC:(r + 1) * CPC]]

        # conv lhsT: wblk[kd_l*32+c', (t*3+i)*36 + kd*4+o] = W_t[o, c']
        #            with kd = kd0_i + kd_l
        wblk = np.zeros((128, 216), np.float32)
        for t in range(2):
            for i, (kd0, nkd) in enumerate(_KD_PASSES):
                for kd_l in range(nkd):
                    kd = kd0 + kd_l
                    rows = slice(kd_l * 32, kd_l * 32 + 32)
                    for o in range(CPC):
                        col = (t * 3 + i) * 36 + kd * 4 + o
                        wblk[rows, col] = Wl[t][o, :]

        # tm[p, t, c, kw, j]; p = kd*4 + o, row 36 = bias (kw=0 only)
        tm = np.zeros((37, 2, CPC, P, P2), np.float32)
        At = np.stack([A / P2, A])                 # x-side carries the 1/81
        bias = np.stack([np.outer(bl[0], rowsum) / P2,
                         np.outer(bl[1], rowsum)])  # (2, 4, 81)
        for kd in range(P):
            for o in range(CPC):
                p = kd * 4 + o
                # tm[p, t, o, kw, j] = At[t, j, kd*9+kw]
                tm[p, :, o, :, :] = At[:, :, kd * P:(kd + 1) * P].transpose(0, 2, 1)
        tm[36, :, :, 0, :] = bias
        tm = tm.reshape(37, 2 * CPC * P * P2)

        in_maps.append({"x": x, "y": y, "wblk": wblk,
                        "tm": np.ascontiguousarray(tm), "ones": ones})
    return in_maps


def kernel(**inputs):
    global last_results
    nc = _build()
    in_maps = _host_prep(**inputs)
    trace = bool(os.environ.get("KERNEL_TRACE"))
    res = run_bass_kernel_spmd(nc, in_maps, core_ids=list(range(N_CORES)),
                               trace=trace)
    last_results = res
    att = np.stack([res.results[r]["att"] for r in range(N_CORES)])
    return att.reshape(1, C, L, L)



# revision 18
# speedup vs baseline: 2.4968x; 2.4968x over previous
"""Trainium2 Bass kernel for nn_Cross_attention_2 (sparse_attention).

Math (B=1, C=32, D=36, H=W=48, P=9):
  xc = conv1x1(x, W_img, b_img)            # per-voxel channel mix
  v  = unfold(xc)                          # (C, L=1024, 81) non-overlapping 9x9 patches
  px = LeakyReLU(v @ (W2@W1)^T + bias)     # the two Linears collapse to A = W2@W1
  att[c] = px[c] @ py[c]^T / 81            # (C, 1024, 1024)

Sharding: channels C=32 split across 8 cores (4 each). Params replicated
(per-core slices precomputed on host). Each core reads full x, y.

Layout/perf notes:
  - Host pretransposes x,y to (D, C, HW) fp16 so every conv input load is a
    2D DMA from a contiguous DRAM block (sprays all 16 SDMA engines; 3D
    transposed APs pin to 4 engines, few-partition transfers to 1).
  - wblk loads first (conv lhsT), then all 24 input tiles on the sync
    HWDGE queue; tm/ones go via gpsimd (SWDGE) so a slow const transfer
    cannot stall the input stream through the 8 round-robin DMA sem lanes.
  - Whole matmul path fp16 (PE streams 1 col/cycle like fp32r, but DMA
    bytes halve); PSUM stays f32; output att is f32.
  - Transform is kw-stacked: xc lives as (109 = 3x36 + bias, g, pd, pw)
    with partition j*36+(kd*4+o) holding kw=3g+j, so each l-chunk needs 3
    accumulating K=109 matmuls instead of 9 K=37 ones (3x fewer PE cols),
    and the rhs is contiguous. The kw gather rides the PSUM->SBUF copy.
  - y is processed first, then x in halves: att rows m0-3 + their output
    DMA start while x pd2/3 still run.
"""

import sys

sys.path.insert(0, "/opt/trn_rl_repo")

import contextlib
import os

import numpy as np

import concourse.bass as bass  # noqa: F401
import concourse.tile as tile
from concourse import bacc, mybir
from concourse.bass_utils import run_bass_kernel_spmd

P = 9
P2 = 81
C = 32
D = 36
HWF = 2304
ND = 4  # pd blocks (D/9)
L = 1024
N_CORES = 8
CPC = 4  # channels per core
KSTACK = 101  # 2 kw-stacked (kd,o) blocks at partitions 0/64 + bias row 100
NG = 5  # kw groups per stack lane (kw = 2g + j)

F32 = mybir.dt.float32
F16 = mybir.dt.float16

_CACHE = {}
last_results = None  # BassKernelResults of the most recent run (for test.py)

_PW_CHUNKS = [(0, 56), (56, 56), (112, 56), (168, 56), (224, 32)]  # pw units
_KD_PASSES = [(0, 4), (4, 4), (8, 1)]  # (kd0, nkd) conv passes


def _build():
    if "nc" in _CACHE:
        return _CACHE["nc"]

    nc = bacc.Bacc("TRN2", target_bir_lowering=False, debug=False,
                   num_devices=N_CORES)
    # x, y pretransposed on host to (D, C, HW)
    x_d = nc.dram_tensor("x", (D, C, HWF), F16, kind="ExternalInput").ap()
    y_d = nc.dram_tensor("y", (D, C, HWF), F16, kind="ExternalInput").ap()
    # wblk: (128, 216) = conv lhsT for (t in 2) x (pass i in 3), 36 cols each
    wblk_d = nc.dram_tensor("wblk", (128, 216), F16, kind="ExternalInput").ap()
    # tm: (101, 2*4*5*81) kw-stacked transform weights
    tm_d = nc.dram_tensor("tm", (KSTACK, 2 * CPC * NG * P2), F16,
                          kind="ExternalInput").ap()
    # fill: bias ones (row 100), zeros for pad rows 36-63 and lane1 g=4
    ones_d = nc.dram_tensor("ones", (1, NG * ND * 256), F16,
                            kind="ExternalInput").ap()
    zer28_d = nc.dram_tensor("zer28", (28, NG * ND * 256), F16,
                             kind="ExternalInput").ap()
    zer36_d = nc.dram_tensor("zer36", (36, ND * 256), F16,
                             kind="ExternalInput").ap()
    att_d = nc.dram_tensor("att", (CPC, L, L), F32, kind="ExternalOutput").ap()

    with tile.TileContext(nc) as tc:
        with contextlib.ExitStack() as ctx:
            consts = ctx.enter_context(tc.tile_pool(name="consts", bufs=1))
            tmpp = ctx.enter_context(tc.tile_pool(name="tmp", bufs=2))
            outp = ctx.enter_context(tc.tile_pool(name="outp", bufs=4))
            cps = ctx.enter_context(tc.tile_pool(name="cps", bufs=2, space="PSUM"))
            tps = ctx.enter_context(tc.tile_pool(name="tps", bufs=3, space="PSUM"))
            aps = ctx.enter_context(tc.tile_pool(name="aps", bufs=3, space="PSUM"))

            # conv weights first: the very first matmul depends on them
            wb_sb = consts.tile([128, 216], F16, tag="wb")
            nc.sync.dma_start(out=wb_sb[:, :], in_=wblk_d[:, :])

            # tm/ones/zeros via SWDGE (gpsimd queue, own sem pool)
            tm_sb = consts.tile([KSTACK, 2 * CPC * NG * P2], F16, tag="tm")
            tmq = 2 * CPC * NG * P2 // 4
            for q in (2, 3, 0, 1):  # t=1 half first
                nc.gpsimd.dma_start(out=tm_sb[:, q * tmq:(q + 1) * tmq],
                                    in_=tm_d[:, q * tmq:(q + 1) * tmq])
            tm_v = tm_sb.rearrange("p (t c g j) -> p t c g j", t=2, c=CPC, g=NG)

            # xc layout: (101, g, pd, pw); p = j*64 + kd*4 + o holds
            # kw = 2g + j; rows 36-63 zero pad, lane1 g=4 zero, row 100 bias
            xc_sb = []
            px_sb = []
            for t in range(2):
                xt = consts.tile([KSTACK, NG, ND, 256], F16, tag=f"xc{t}")
                nc.gpsimd.dma_start(
                    out=xt[100:101, :, :, :],
                    in_=ones_d.rearrange("p (g d w) -> p g d w", g=NG, d=ND))
                nc.gpsimd.dma_start(
                    out=xt[36:64, :, :, :],
                    in_=zer28_d.rearrange("p (g d w) -> p g d w", g=NG, d=ND))
                nc.gpsimd.dma_start(
                    out=xt[64:100, 4, :, :],
                    in_=zer36_d.rearrange("p (d w) -> p d w", d=ND))
                xc_sb.append(xt)
                px_sb.append([consts.tile([P2, L], F16, tag=f"px{t}{c}",
                                          name=f"px{t}{c}")
                              for c in range(CPC)])

            # ---- all input loads: y then x ----
            xb = {}
            for t in (1, 0):
                src = y_d if t == 1 else x_d
                for pd in range(ND):
                    for i, (kd0, nkd) in enumerate(_KD_PASSES):
                        kp = 32 * nkd
                        xbt = consts.tile([kp, HWF], F16, tag=f"xb{t}{pd}{i}")
                        rows = src[pd * P + kd0: pd * P + kd0 + nkd, :, :]
                        nc.sync.dma_start(
                            out=xbt[:, :],
                            in_=rows.rearrange("a b c -> (a b) c"))
                        xb[(t, pd, i)] = xbt

            def conv_pd(t, pd):
                for ci, (pw0, pwn) in enumerate(_PW_CHUNKS):
                    hn = pwn * P
                    ps = cps.tile([36, 504], F32, tag="cps")
                    for i, (kd0, nkd) in enumerate(_KD_PASSES):
                        kp = 32 * nkd
                        lhs = wb_sb[0:kp,
                                    (t * 3 + i) * 36: (t * 3 + i + 1) * 36]
                        nc.tensor.matmul(
                            ps[:, :hn], lhs,
                            xb[(t, pd, i)][:, pw0 * P: pw0 * P + hn],
                            start=(i == 0), stop=(i == 2))
                    # kw-stack gather in the PSUM->SBUF copy: kw = 2g + j goes
                    # to partitions [j*64, j*64+36) at free idx (g, pd, pw)
                    # (engine APs need 32-aligned partition bases: 0 and 64)
                    srck = ps[:, :hn].rearrange("p (w k) -> p k w", k=P)
                    for j in range(2):
                        gn = NG - j  # j=0: kw 0,2,4,6,8; j=1: kw 1,3,5,7
                        dst = xc_sb[t][j * 64: j * 64 + 36, 0:gn, pd,
                                       pw0: pw0 + pwn]
                        src = srck[:, j: j + 2 * gn - 1: 2, :]
                        if (ci * 2 + j) % 2 == 0:
                            nc.vector.tensor_copy(out=dst, in_=src)
                        else:
                            nc.scalar.copy(out=dst, in_=src)

            def transform(t, c, ch):
                # z = sum_g TM[t,c,g].T @ xc5[:, g, pd-pair]; px = LeakyReLU(z)
                zp = tps.tile([P2, 512], F32, tag="tps")
                for g in range(NG):
                    rhs = xc_sb[t][:, g, 2 * ch: 2 * ch + 2, :]
                    nc.tensor.matmul(
                        zp[:, :], tm_v[:, t, c, g, :], rhs,
                        start=(g == 0), stop=(g == NG - 1))
                # LeakyReLU(z) = max(0.2*z, z)
                zm = tmpp.tile([P2, 512], F32, tag="zm")
                nc.scalar.mul(zm[:, :], zp[:, :], 0.2)
                nc.vector.tensor_tensor(
                    out=px_sb[t][c][:, ch * 512: ch * 512 + 512],
                    in0=zp[:, :], in1=zm[:, :], op=mybir.AluOpType.max)

            def att(c, m):
                ob = outp.tile([128, L], F32, tag="ob")
                for nch in range(2):
                    ap_ = aps.tile([128, 512], F32, tag="aps")
                    nc.tensor.matmul(
                        ap_[:, :],
                        px_sb[0][c][:, m * 128: m * 128 + 128],
                        px_sb[1][c][:, nch * 512: nch * 512 + 512],
                        start=True, stop=True)
                    # split the PSUM->SBUF copy across both engines
                    dst = ob[:, nch * 512: nch * 512 + 512]
                    nc.vector.tensor_copy(out=dst[:, 0:256], in_=ap_[:, 0:256])
                    nc.scalar.copy(out=dst[:, 256:512], in_=ap_[:, 256:512])
                nc.sync.dma_start(
                    out=att_d[c, m * 128: m * 128 + 128, :], in_=ob[:, :])

            # ---- y: conv + transform for all channels ----
            conv_pd(1, 0)
            conv_pd(1, 1)
            for c in range(CPC):
                transform(1, c, 0)
            conv_pd(1, 2)
            conv_pd(1, 3)
            for c in range(CPC):
                transform(1, c, 1)

            # ---- x first half -> att rows m0-3 (output DMA starts early) ----
            conv_pd(0, 0)
            conv_pd(0, 1)
            for c in range(CPC):
                transform(0, c, 0)
                for m in range(4):
                    att(c, m)

            # ---- x second half -> att rows m4-7 ----
            conv_pd(0, 2)
            conv_pd(0, 3)
            for c in range(CPC):
                transform(0, c, 1)
                for m in range(4, 8):
                    att(c, m)

    nc.compile()
    _CACHE["nc"] = nc
    return nc


def _host_prep(x, y, W_img, b_img, W_fea, b_fea, W1, W2):
    """Build per-core wblk / tm arrays. Returns in_maps list."""
    x = np.asarray(x, np.float32).reshape(C, D, HWF).transpose(1, 0, 2)
    y = np.asarray(y, np.float32).reshape(C, D, HWF).transpose(1, 0, 2)
    x = np.ascontiguousarray(x, np.float16)
    y = np.ascontiguousarray(y, np.float16)
    W_img = np.asarray(W_img, np.float32)
    b_img = np.asarray(b_img, np.float32)
    W_fea = np.asarray(W_fea, np.float32)
    b_fea = np.asarray(b_fea, np.float32)
    A = np.asarray(W2, np.float32) @ np.asarray(W1, np.float32)  # (81, 81)
    rowsum = A.sum(axis=1)  # (81,)
    ones = np.ones((1, NG * ND * 256), np.float16)
    zer28 = np.zeros((28, NG * ND * 256), np.float16)
    zer36 = np.zeros((36, ND * 256), np.float16)

    in_maps = []
    for r in range(N_CORES):
        Wl = [W_img[r * CPC:(r + 1) * CPC, :], W_fea[r * CPC:(r + 1) * CPC, :]]
        bl = [b_img[r * CPC:(r + 1) * CPC], b_fea[r * CPC:(r + 1) * CPC]]

        # conv lhsT: wblk[kd_l*32+c', (t*3+i)*36 + kd*4+o] = W_t[o, c']
        #            with kd = kd0_i + kd_l
        wblk = np.zeros((128, 216), np.float32)
        for t in range(2):
            for i, (kd0, nkd) in enumerate(_KD_PASSES):
                for kd_l in range(nkd):
                    kd = kd0 + kd_l
                    rows = slice(kd_l * 32, kd_l * 32 + 32)
                    for o in range(CPC):
                        col = (t * 3 + i) * 36 + kd * 4 + o
                        wblk[rows, col] = Wl[t][o, :]

        # tm[p, t, c, g, j]; p = jj*64 + kd*4 + o holds kw = 2g + jj,
        # row 100 = bias (pass g=0 only)
        tm = np.zeros((KSTACK, 2, CPC, NG, P2), np.float32)
        At = np.stack([A / P2, A])                 # x-side carries the 1/81
        bias = np.stack([np.outer(bl[0], rowsum) / P2,
                         np.outer(bl[1], rowsum)])  # (2, 4, 81)
        for jj in range(2):
            for kd in range(P):
                for o in range(CPC):
                    p = jj * 64 + kd * 4 + o
                    for g in range(NG - jj):
                        kw = 2 * g + jj
                        # tm[p, t, o, g, :] = At[t, :, kd*9+kw]
                        tm[p, :, o, g, :] = At[:, :, kd * P + kw]
        tm[100, :, :, 0, :] = bias
        tm = tm.reshape(KSTACK, 2 * CPC * NG * P2)

        in_maps.append({"x": x, "y": y,
                        "wblk": wblk.astype(np.float16),
                        "tm": np.ascontiguousarray(tm.astype(np.float16)),
                        "ones": ones, "zer28": zer28, "zer36": zer36})
    return in_maps


def kernel(**inputs):
    global last_results
    nc = _build()
    in_maps = _host_prep(**inputs)
    trace = bool(os.environ.get("KERNEL_TRACE"))
    res = run_bass_kernel_spmd(nc, in_maps, core_ids=list(range(N_CORES)),
                               trace=trace)
    last_results = res
    att = np.stack([res.results[r]["att"] for r in range(N_CORES)])
    return att.reshape(1, C, L, L)
